# revision 19
# baseline (speedup 1.0000x reference)
"""Multi-head cross-attention (softmax over the QUERY axis) on 8 TRN2 cores.

Sharding: core c handles batch b = c // 4 and head-group hg = c % 4
(4 heads of 64 dims each = 256 columns of Wq / Wkv-K / Wkv-V, 256 rows
of Wo).  Each core computes a partial final output (its head group's
contribution to  out @ Wo); the host sums the 4 partials per batch.
bo is fed only to the hg == 0 cores so it is added exactly once.

Per-core device kernel (single Bass SPMD program, data-driven):
  phase 1: transpose X / Xe on the PE (exact transpose mode), project
           Q^T, K^T  [256, 2048]  and V [2048, 256]  with fp32r matmuls
  phase 2: heads processed in PAIRS sharing the PE array via tile
           position inference (even head: SBUF partitions 0-63 -> T0,
           odd head: partitions 64-127 -> T8 for the 64-contraction
           score matmuls; attnV col-tiles T0/T1 write the two PSUM
           partition halves of one [128, 2048] out^T accumulator).
           scores S^T[k, q]; one ACT Exp pass evacuates PSUM -> SBUF
           and emits per-key sum(exp) via accum_out; 1/sum is folded
           into V (per-partition scale).
  phase 3: Y[t, :] = O^T.T @ Wo (+ bo via a K=1 ones-row matmul)
"""

import os
import sys

for _p in ("/opt/trn_rl_repo", "/root/.axon_site/_ro/trn_rl_repo"):
    if os.path.isdir(_p) and _p not in sys.path:
        sys.path.insert(0, _p)

import numpy as np

import concourse.bass as bass  # noqa: E402
import concourse.mybir as mybir  # noqa: E402
import concourse.tile as tile  # noqa: E402
from concourse import bacc  # noqa: E402
from concourse.bass_utils import run_bass_kernel_spmd  # noqa: E402
from concourse.masks import make_identity  # noqa: E402

B, T, E, H, D = 2, 2048, 1024, 16, 64
HG = 4                # heads per core
DL = HG * D           # 256 local head-dim columns
NCORES = 8
NT = T // 128         # 16 row tiles
NE = E // 128         # 8 contraction blocks
F32 = mybir.dt.float32
F32R = mybir.dt.float32r
BF16 = mybir.dt.bfloat16
AF = mybir.ActivationFunctionType


def _r(ap):
    return ap.bitcast(F32R)


def build_program():
    nc = bacc.Bacc()
    x_d = nc.dram_tensor("x", [T, E], F32, kind="ExternalInput")
    xe_d = nc.dram_tensor("xe", [T, E], F32, kind="ExternalInput")
    wq_d = nc.dram_tensor("wq", [E, DL], F32, kind="ExternalInput")
    wk_d = nc.dram_tensor("wk", [E, DL], F32, kind="ExternalInput")
    wv_d = nc.dram_tensor("wv", [E, DL], F32, kind="ExternalInput")
    wo_d = nc.dram_tensor("wo", [DL, E], F32, kind="ExternalInput")
    bq_d = nc.dram_tensor("bq", [DL], F32, kind="ExternalInput")
    bk_d = nc.dram_tensor("bk", [DL], F32, kind="ExternalInput")
    bv_d = nc.dram_tensor("bv", [DL], F32, kind="ExternalInput")
    bo_d = nc.dram_tensor("bo", [E], F32, kind="ExternalInput")
    y_d = nc.dram_tensor("y", [T, E], F32, kind="ExternalOutput")

    with tile.TileContext(nc) as tc:
        with (
            tc.tile_pool(name="const", bufs=1) as cons,
            tc.tile_pool(name="persist", bufs=1) as pers,
            tc.tile_pool(name="wqkv", bufs=1) as wqkvp,
        ):
            # ---- constants first (gpsimd), weights later on the sync q ---
            ones = cons.tile([128, 128], F32, tag="ones")
            nc.gpsimd.memset(ones[:], 1.0)
            nc.vector.tensor_copy(_r(ones[:]), ones[:])
            ident = cons.tile([128, 128], F32, tag="ident")
            make_identity(nc, ident[:])
            zeros = cons.tile([128, 64], F32, tag="zeros")
            nc.gpsimd.memset(zeros[:], 0.0)
            bq_sb = cons.tile([128, 2], F32, tag="bq")
            bk_sb = cons.tile([128, 2], F32, tag="bk")
            bv_sb = cons.tile([128, 2], F32, tag="bv")
            nc.gpsimd.dma_start(out=bq_sb[:], in_=bq_d[:].rearrange("(a p) -> p a", p=128))
            nc.gpsimd.dma_start(out=bk_sb[:], in_=bk_d[:].rearrange("(a p) -> p a", p=128))
            nc.gpsimd.dma_start(out=bv_sb[:], in_=bv_d[:].rearrange("(a p) -> p a", p=128))
            wq_sb = wqkvp.tile([128, NE, DL], F32, tag="wq")
            wk_sb = wqkvp.tile([128, NE, DL], F32, tag="wk")
            wv_sb = wqkvp.tile([128, NE, DL], F32, tag="wv")
            wo_sb = cons.tile([128, 2, E], F32, tag="wo")
            bo_r = cons.tile([128, E], F32, tag="bo_r")

            def load_weight(w_d, w_sb, nb, nn, wtmpp):
                wr = w_d[:].rearrange("(a p) n -> p a n", p=128)
                for blk in range(nb):
                    wtmp = wtmpp.tile([128, E], F32, tag="wtmp", name=f"wt{blk}")
                    nc.sync.dma_start(out=wtmp[:, 0:nn], in_=wr[:, blk, :])
                    nc.vector.tensor_copy(_r(w_sb[:, blk, :]), wtmp[:, 0:nn])

            # ---- persistent activations ----------------------------------
            QT = pers.tile([128, 2, T], F32, tag="QT")   # Q^T: part = d' % 128, blk = d' // 128
            KT = pers.tile([128, 2, T], F32, tag="KT")
            Vt = pers.tile([128, NT, DL], F32, tag="V")  # V:   part = t % 128, blk = t // 128
            OT = pers.tile([128, 2, T], F32, tag="OT")   # out^T, same layout as QT

            # ================= phase 1 helpers ============================
            def transpose_in(src_d, xT, xrowp, ps1):
                for tt in range(NT):
                    xrow = xrowp.tile([128, E], F32, tag="xrow")
                    nc.sync.dma_start(out=xrow[:], in_=src_d[tt * 128:(tt + 1) * 128, :])
                    pt = ps1.tile([128, 1024], F32, tag="s", name=f"tp{tt}")
                    for eb in range(NE):
                        nc.tensor.transpose(pt[:, eb * 128:(eb + 1) * 128],
                                            xrow[:, eb * 128:(eb + 1) * 128], ident[:])
                    dst = _r(xT[:, 0:NE, tt * 128:(tt + 1) * 128])
                    if tt % 2 == 0:
                        nc.vector.tensor_copy(dst, pt[:].rearrange("p (b n) -> p b n", b=NE))
                    else:
                        nc.scalar.activation(dst, pt[:].rearrange("p (b n) -> p b n", b=NE), AF.Copy)

            def project(w_sb, b_sb, xT, dstT, a, ps1, only_tp=None):
                # dstT[:, a, t] = sum_eb w[:, eb, a*128:+128].T @ xT[:, eb, t] + b
                for tp in ((only_tp,) if only_tp is not None else (0, 1)):
                    pts = [ps1.tile([128, 512], F32, tag="s", name=f"pt{a}_{tp}_{_i}")
                           for _i in range(2)]
                    for eb in range(NE):
                        for i in range(2):
                            tc4 = tp * 2 + i
                            nc.tensor.matmul(
                                pts[i][:],
                                lhsT=_r(w_sb[:, eb, a * 128:(a + 1) * 128]),
                                rhs=_r(xT[:, eb, tc4 * 512:(tc4 + 1) * 512]),
                                start=(eb == 0),
                                stop=(eb == NE - 1),
                            )
                    for i in range(2):
                        tc4 = tp * 2 + i
                        nc.scalar.activation(
                            _r(dstT[:, a, tc4 * 512:(tc4 + 1) * 512]), pts[i][:],
                            AF.Identity, bias=b_sb[:, a:a + 1],
                        )

            def project_v(a, xeT, vttp, ps1, only_tp=None):
                # V^T block a, then transpose into Vt [t, d']
                vtt = vttp.tile([128, T], F32, tag="vtt", name=f"vtt{a}_{only_tp}")
                for tp in ((only_tp,) if only_tp is not None else (0, 1)):
                    pts = [ps1.tile([128, 512], F32, tag="s", name=f"pv{a}_{tp}_{_i}")
                           for _i in range(2)]
                    for eb in range(NE):
                        for i in range(2):
                            tc4 = tp * 2 + i
                            nc.tensor.matmul(
                                pts[i][:],
                                lhsT=_r(wv_sb[:, eb, a * 128:(a + 1) * 128]),
                                rhs=_r(xeT[:, eb, tc4 * 512:(tc4 + 1) * 512]),
                                start=(eb == 0),
                                stop=(eb == NE - 1),
                            )
                    for i in range(2):
                        tc4 = tp * 2 + i
                        nc.scalar.activation(
                            vtt[:, tc4 * 512:(tc4 + 1) * 512], pts[i][:],
                            AF.Identity, bias=bv_sb[:, a:a + 1],
                        )
                for tg in ((only_tp,) if only_tp is not None else (0, 1)):
                    pt = ps1.tile([128, 1024], F32, tag="s", name=f"vt{a}_{tg}")
                    for j in range(NE):
                        tt = tg * NE + j
                        nc.tensor.transpose(pt[:, j * 128:(j + 1) * 128],
                                            vtt[:, tt * 128:(tt + 1) * 128], ident[:])
                    nc.vector.tensor_copy(
                        _r(Vt[:, tg * NE:(tg + 1) * NE, a * 128:(a + 1) * 128]),
                        pt[:].rearrange("p (b n) -> p b n", b=NE))

            # ================= phase 2: one head PAIR =====================
            def attention_pair(hb, pss, pso, pbufp, smallp, kpp, vmvp, fillers=None):
                # Heads (2*hb, 2*hb+1) processed together with FULL-array
                # matmuls: per 64-key subtile kts, a block-diagonal K
                # stationary [128 d-pair, 128 (head,key)-slots] yields both
                # heads' scores in one K=128 matmul; a block V' stationary
                # [128 (head,key), 128 d-pair] does attnV the same way.
                # Key slots: head-even -> partitions pi(kts) (the partition
                # range where V[t] for this subtile lives), head-odd -> the
                # complement (its V slice is DMA-moved across partitions).
                po = pso.tile([128, T], F32, tag="po", name=f"po{hb}")
                he, hod = 2 * hb, 2 * hb + 1
                prev = None  # deferred attnV of the previous subtile
                for kts in range(2 * NT):
                    tt, pi = kts // 2, 64 * (kts % 2)
                    pj = 64 - pi  # complement offset
                    t0 = kts * 64 % 128
                    # --- block-diagonal K stationary ---
                    kp = kpp.tile([128, 128], F32, tag="kp", name=f"kp{hb}_{kts}")
                    nc.vector.tensor_copy(_r(kp[0:64, pj:pj + 64]), zeros[0:64, :])
                    nc.vector.tensor_copy(_r(kp[64:128, pi:pi + 64]), zeros[64:128, :])
                    nc.vector.tensor_copy(_r(kp[0:64, pi:pi + 64]),
                                          KT[0:64, hb, kts * 64:(kts + 1) * 64])
                    nc.vector.tensor_copy(_r(kp[64:128, pj:pj + 64]),
                                          KT[64:128, hb, kts * 64:(kts + 1) * 64])
                    # --- V slice for the odd head moved to the complement ---
                    vmv = vmvp.tile([128, 64], F32, tag="vmv", name=f"vm{hb}_{kts}")
                    nc.sync.dma_start(out=vmv[pj:pj + 64, :],
                                      in_=Vt[t0:t0 + 64, tt, hod * 64:hod * 64 + 64])
                    # --- scores + exp (both heads per instruction) ---
                    pP = pbufp.tile([128, T], BF16, tag="P", name=f"pP{hb}_{kts}")
                    sums = smallp.tile([128, 2], F32, tag="sums", name=f"sm{hb}_{kts}")
                    for half in range(2):
                        ps = pss.tile([128, 1024], F32, tag="s", name=f"ps{hb}_{kts}_{half}")
                        for qc in range(2):
                            q0 = half * 1024 + qc * 512
                            nc.tensor.matmul(
                                ps[:, qc * 512:(qc + 1) * 512],
                                lhsT=_r(kp[:]),
                                rhs=_r(QT[:, hb, q0:q0 + 512]),
                                start=True, stop=True,
                            )
                        nc.scalar.activation(
                            pP[:, half * 1024:(half + 1) * 1024], ps[:],
                            AF.Exp, scale=0.125,
                        )
                        nc.vector.reduce_sum(
                            sums[:, half:half + 1],
                            pP[:, half * 1024:(half + 1) * 1024],
                            axis=mybir.AxisListType.X,
                        )
                    # deferred attnV of the previous subtile keeps the PE busy
                    if prev is not None:
                        emit_attnv(*prev)
                    # --- 1/sumexp, block V' stationary (bf16) ---
                    stot = smallp.tile([128, 1], F32, tag="stot", name=f"st{hb}_{kts}")
                    nc.vector.tensor_add(stot[:], sums[:, 0:1], sums[:, 1:2])
                    rec = smallp.tile([128, 1], F32, tag="rec", name=f"rc{hb}_{kts}")
                    nc.vector.reciprocal(rec[:], stot[:])
                    vp = smallp.tile([128, 128], BF16, tag="vp", name=f"vp{hb}_{kts}")
                    nc.gpsimd.memset(vp[pi:pi + 64, 64:128], 0.0)
                    nc.gpsimd.memset(vp[pj:pj + 64, 0:64], 0.0)
                    nc.vector.tensor_scalar_mul(
                        vp[pi:pi + 64, 0:64],
                        Vt[t0:t0 + 64, tt, he * 64:he * 64 + 64],
                        rec[pi:pi + 64, :])
                    nc.vector.tensor_scalar_mul(
                        vp[pj:pj + 64, 64:128], vmv[pj:pj + 64, :],
                        rec[pj:pj + 64, :])
                    prev = (po, vp, pP, kts)
                    if fillers and kts in fillers:
                        fillers[kts]()
                emit_attnv(*prev)
                nc.vector.tensor_copy(_r(OT[:, hb, :]), po[:])

            def emit_attnv(po, vp, pP, kts):
                for qc4 in range(4):
                    nc.tensor.matmul(
                        po[:, qc4 * 512:(qc4 + 1) * 512],
                        lhsT=vp[:],
                        rhs=pP[:, qc4 * 512:(qc4 + 1) * 512],
                        start=(kts == 0), stop=(kts == 2 * NT - 1),
                    )

            # ================= program order ==============================
            with (
                tc.tile_pool(name="xtp", bufs=1) as xtp,
                tc.tile_pool(name="vtt", bufs=1) as vttp,
                tc.tile_pool(name="ps_s", bufs=2, space="PSUM") as pss,
                tc.tile_pool(name="ps_o", bufs=1, space="PSUM") as pso,
                tc.tile_pool(name="pbuf", bufs=3) as pbufp,
                tc.tile_pool(name="small", bufs=4) as smallp,
                tc.tile_pool(name="kpp", bufs=3) as kpp,
                tc.tile_pool(name="vmv", bufs=4) as vmvp,
            ):
                with (
                    tc.tile_pool(name="xrow", bufs=2) as xrowp,
                    tc.tile_pool(name="wtmp", bufs=2) as wtmpp,
                ):
                    xT = xtp.tile([128, NE, T], F32, tag="xt")
                    load_weight(wq_d, wq_sb, NE, DL, wtmpp)
                    transpose_in(x_d, xT, xrowp, pss)
                    load_weight(wk_d, wk_sb, NE, DL, wtmpp)
                    load_weight(wv_d, wv_sb, NE, DL, wtmpp)
                    project(wq_sb, bq_sb, xT, QT, 0, pss)
                    project(wq_sb, bq_sb, xT, QT, 1, pss)

                    xeT = xtp.tile([128, NE, T], F32, tag="xt")
                    transpose_in(xe_d, xeT, xrowp, pss)
                    load_weight(wo_d, wo_sb, 2, E, wtmpp)
                    botmp = wtmpp.tile([128, E], F32, tag="wtmp")
                    nc.sync.dma_start(out=botmp[0:1, :], in_=bo_d[:].rearrange("(o e) -> o e", o=1))
                    nc.vector.tensor_copy(_r(bo_r[0:1, :]), botmp[0:1, :])
                    project(wk_sb, bk_sb, xeT, KT, 0, pss)
                    project_v(0, xeT, vttp, pss)
                    # pair 0 attention; the tail of pair 0 interleaves the
                    # first half of the a=1 projections as fillers, the rest
                    # lands inside pair 1's early iterations
                    attention_pair(0, pss, pso, pbufp, smallp, kpp, vmvp, fillers={
                        19: lambda: project(wk_sb, bk_sb, xeT, KT, 1, pss, only_tp=0),
                        25: lambda: project_v(1, xeT, vttp, pss, only_tp=0),
                    })
                    attention_pair(1, pss, pso, pbufp, smallp, kpp, vmvp, fillers={
                        3: lambda: project(wk_sb, bk_sb, xeT, KT, 1, pss, only_tp=1),
                        9: lambda: project_v(1, xeT, vttp, pss, only_tp=1),
                    })

            # ================= phase 3: output projection =================
            with (
                tc.tile_pool(name="ps_y", bufs=4, space="PSUM") as psy,
                tc.tile_pool(name="ysb", bufs=2) as ysbp,
            ):
                for tt in range(NT):
                    ysb = ysbp.tile([128, E], F32, tag="ysb")
                    pys = [psy.tile([128, 512], F32, tag="py", name=f"py{tt}_{_i}")
                           for _i in range(2)]
                    for a in range(2):
                        for ec in range(2):
                            nc.tensor.matmul(
                                pys[ec][:],
                                lhsT=_r(OT[:, a, tt * 128:(tt + 1) * 128]),
                                rhs=_r(wo_sb[:, a, ec * 512:(ec + 1) * 512]),
                                start=(a == 0), stop=False,
                            )
                    for ec in range(2):
                        nc.tensor.matmul(
                            pys[ec][:],
                            lhsT=_r(ones[0:1, 0:128]),
                            rhs=_r(bo_r[0:1, ec * 512:(ec + 1) * 512]),
                            start=False, stop=True,
                        )
                        nc.vector.tensor_copy(ysb[:, ec * 512:(ec + 1) * 512], pys[ec][:])
                    nc.sync.dma_start(out=y_d[tt * 128:(tt + 1) * 128, :], in_=ysb[:])

    nc.compile()
    return nc


_NC_CACHE = []


def _get_program():
    if not _NC_CACHE:
        _NC_CACHE.append(build_program())
    return _NC_CACHE[0]


def make_in_maps(input, encoded_features, Wq, bq, Wkv, bkv, Wo, bo):
    input = np.asarray(input, dtype=np.float32)
    encoded_features = np.asarray(encoded_features, dtype=np.float32)
    Wq = np.asarray(Wq, dtype=np.float32)
    bq = np.asarray(bq, dtype=np.float32)
    Wkv = np.asarray(Wkv, dtype=np.float32)
    bkv = np.asarray(bkv, dtype=np.float32)
    Wo = np.asarray(Wo, dtype=np.float32)
    bo = np.asarray(bo, dtype=np.float32)
    in_maps = []
    for c in range(NCORES):
        b, hg = c // 4, c % 4
        s = slice(hg * DL, (hg + 1) * DL)
        in_maps.append({
            "x": np.ascontiguousarray(input[b]),
            "xe": np.ascontiguousarray(encoded_features[b]),
            "wq": np.ascontiguousarray(Wq[:, s]),
            "wk": np.ascontiguousarray(Wkv[:, s]),
            "wv": np.ascontiguousarray(Wkv[:, E + hg * DL:E + (hg + 1) * DL]),
            "wo": np.ascontiguousarray(Wo[s, :]),
            "bq": np.ascontiguousarray(bq[s]),
            "bk": np.ascontiguousarray(bkv[s]),
            "bv": np.ascontiguousarray(bkv[E + hg * DL:E + (hg + 1) * DL]),
            "bo": (bo if hg == 0 else np.zeros_like(bo)),
        })
    return in_maps


def combine_outputs(results):
    out = np.zeros((B, T, E), dtype=np.float32)
    for c in range(NCORES):
        out[c // 4] += results[c]["y"]
    return out


def kernel(input, encoded_features, Wq, bq, Wkv, bkv, Wo, bo, _trace=False):
    nc = _get_program()
    in_maps = make_in_maps(input, encoded_features, Wq, bq, Wkv, bkv, Wo, bo)
    res = run_bass_kernel_spmd(nc, in_maps, list(range(NCORES)), trace=_trace)
    out = combine_outputs(res.results)
    if _trace:
        kernel.last_exec_time_ns = res.exec_time_ns
        kernel.last_results = res
    return out


if __name__ == "__main__":
    rng = np.random.default_rng(0)
    inputs = {
        "input": rng.standard_normal((B, T, E), dtype=np.float32),
        "encoded_features": rng.standard_normal((B, T, E), dtype=np.float32),
        "Wq": (rng.standard_normal((E, E), dtype=np.float32) / 32.0),
        "bq": np.zeros(E, np.float32),
        "Wkv": (rng.standard_normal((E, 2 * E), dtype=np.float32) / 32.0),
        "bkv": np.zeros(2 * E, np.float32),
        "Wo": (rng.standard_normal((E, E), dtype=np.float32) / 32.0),
        "bo": np.zeros(E, np.float32),
    }
    out = kernel(**inputs)
    print("out shape", out.shape, out.dtype)


# revision 20
# speedup vs baseline: 1.1732x; 1.1732x over previous
"""Multi-head cross-attention (softmax over the QUERY axis) on 8 TRN2 cores.

Sharding: core c handles batch b = c // 4 and head-group hg = c % 4
(4 heads of 64 dims each = 256 columns of Wq / Wkv-K / Wkv-V, 256 rows
of Wo).  Each core computes a partial final output (its head group's
contribution to  out @ Wo); the host sums the 4 partials per batch.
bo is fed only to the hg == 0 cores so it is added exactly once.

Per-core device kernel (single Bass SPMD program, data-driven):
  phase 1: transpose X / Xe on the PE (exact transpose mode), project
           Q^T, K^T  [256, 2048]  and V [2048, 256]  with fp32r matmuls
  phase 2: heads processed in PAIRS sharing the PE array via tile
           position inference (even head: SBUF partitions 0-63 -> T0,
           odd head: partitions 64-127 -> T8 for the 64-contraction
           score matmuls; attnV col-tiles T0/T1 write the two PSUM
           partition halves of one [128, 2048] out^T accumulator).
           scores S^T[k, q]; one ACT Exp pass evacuates PSUM -> SBUF
           and emits per-key sum(exp) via accum_out; 1/sum is folded
           into V (per-partition scale).
  phase 3: Y[t, :] = O^T.T @ Wo (+ bo via a K=1 ones-row matmul)
"""

import os
import sys

for _p in ("/opt/trn_rl_repo", "/root/.axon_site/_ro/trn_rl_repo"):
    if os.path.isdir(_p) and _p not in sys.path:
        sys.path.insert(0, _p)

import numpy as np

import concourse.bass as bass  # noqa: E402
import concourse.mybir as mybir  # noqa: E402
import concourse.tile as tile  # noqa: E402
from concourse import bacc  # noqa: E402
from concourse.bass_utils import run_bass_kernel_spmd  # noqa: E402
from concourse.masks import make_identity  # noqa: E402

B, T, E, H, D = 2, 2048, 1024, 16, 64
HG = 4                # heads per core
DL = HG * D           # 256 local head-dim columns
NCORES = 8
NT = T // 128         # 16 row tiles
NE = E // 128         # 8 contraction blocks
F32 = mybir.dt.float32
F32R = mybir.dt.float32r
BF16 = mybir.dt.bfloat16
AF = mybir.ActivationFunctionType


def _r(ap):
    return ap.bitcast(F32R)


def build_program():
    nc = bacc.Bacc()
    x_d = nc.dram_tensor("x", [T, E], F32, kind="ExternalInput")
    xe_d = nc.dram_tensor("xe", [T, E], F32, kind="ExternalInput")
    wq_d = nc.dram_tensor("wq", [E, DL], F32, kind="ExternalInput")
    wk_d = nc.dram_tensor("wk", [E, DL], F32, kind="ExternalInput")
    wv_d = nc.dram_tensor("wv", [E, DL], F32, kind="ExternalInput")
    wo_d = nc.dram_tensor("wo", [DL, E], F32, kind="ExternalInput")
    bq_d = nc.dram_tensor("bq", [DL], F32, kind="ExternalInput")
    bk_d = nc.dram_tensor("bk", [DL], F32, kind="ExternalInput")
    bv_d = nc.dram_tensor("bv", [DL], F32, kind="ExternalInput")
    bo_d = nc.dram_tensor("bo", [E], F32, kind="ExternalInput")
    y_d = nc.dram_tensor("y", [T, E], F32, kind="ExternalOutput")

    with tile.TileContext(nc) as tc:
        with (
            tc.tile_pool(name="const", bufs=1) as cons,
            tc.tile_pool(name="persist", bufs=1) as pers,
            tc.tile_pool(name="wqkv", bufs=1) as wqkvp,
        ):
            # ---- constants first (gpsimd), weights later on the sync q ---
            ones = cons.tile([128, 128], F32, tag="ones")
            nc.gpsimd.memset(ones[:], 1.0)
            nc.vector.tensor_copy(_r(ones[:]), ones[:])
            ident = cons.tile([128, 128], F32, tag="ident")
            make_identity(nc, ident[:])
            zeros = cons.tile([128, 64], F32, tag="zeros")
            nc.gpsimd.memset(zeros[:], 0.0)
            bq_sb = cons.tile([128, 2], F32, tag="bq")
            bk_sb = cons.tile([128, 2], F32, tag="bk")
            bv_sb = cons.tile([128, 2], F32, tag="bv")
            nc.gpsimd.dma_start(out=bq_sb[:], in_=bq_d[:].rearrange("(a p) -> p a", p=128))
            nc.gpsimd.dma_start(out=bk_sb[:], in_=bk_d[:].rearrange("(a p) -> p a", p=128))
            nc.gpsimd.dma_start(out=bv_sb[:], in_=bv_d[:].rearrange("(a p) -> p a", p=128))
            wq_sb = wqkvp.tile([128, NE, DL], F32, tag="wq")
            wk_sb = wqkvp.tile([128, NE, DL], F32, tag="wk")
            wv_sb = wqkvp.tile([128, NE, DL], F32, tag="wv")
            wo_sb = cons.tile([128, 2, E], F32, tag="wo")
            bo_r = cons.tile([128, E], F32, tag="bo_r")

            def load_weight(w_d, w_sb, nb, nn, wtmpp):
                wr = w_d[:].rearrange("(a p) n -> p a n", p=128)
                for blk in range(nb):
                    wtmp = wtmpp.tile([128, E], F32, tag="wtmp", name=f"wt{blk}")
                    nc.sync.dma_start(out=wtmp[:, 0:nn], in_=wr[:, blk, :])
                    nc.vector.tensor_copy(_r(w_sb[:, blk, :]), wtmp[:, 0:nn])

            # ---- persistent activations ----------------------------------
            QT = pers.tile([128, 2, T], F32, tag="QT")   # Q^T: part = d' % 128, blk = d' // 128
            KT = pers.tile([128, 2, T], F32, tag="KT")
            Vt = pers.tile([128, NT, DL], F32, tag="V")  # V:   part = t % 128, blk = t // 128
            OT = pers.tile([128, 2, T], F32, tag="OT")   # out^T, same layout as QT

            # ================= phase 1 helpers ============================
            def transpose_in(src_d, xT, xrowp, ps1):
                for tt in range(NT):
                    xrow = xrowp.tile([128, E], F32, tag="xrow")
                    nc.sync.dma_start(out=xrow[:], in_=src_d[tt * 128:(tt + 1) * 128, :])
                    pt = ps1.tile([128, 1024], F32, tag="s", name=f"tp{tt}")
                    for eb in range(NE):
                        nc.tensor.transpose(pt[:, eb * 128:(eb + 1) * 128],
                                            xrow[:, eb * 128:(eb + 1) * 128], ident[:])
                    dst = _r(xT[:, 0:NE, tt * 128:(tt + 1) * 128])
                    if tt % 2 == 0:
                        nc.vector.tensor_copy(dst, pt[:].rearrange("p (b n) -> p b n", b=NE))
                    else:
                        nc.scalar.activation(dst, pt[:].rearrange("p (b n) -> p b n", b=NE), AF.Copy)

            def project(w_sb, b_sb, xT, dstT, a, ps1, only_tp=None):
                # dstT[:, a, t] = sum_eb w[:, eb, a*128:+128].T @ xT[:, eb, t] + b
                for tp in ((only_tp,) if only_tp is not None else (0, 1)):
                    pts = [ps1.tile([128, 512], F32, tag="s", name=f"pt{a}_{tp}_{_i}")
                           for _i in range(2)]
                    for eb in range(NE):
                        for i in range(2):
                            tc4 = tp * 2 + i
                            nc.tensor.matmul(
                                pts[i][:],
                                lhsT=_r(w_sb[:, eb, a * 128:(a + 1) * 128]),
                                rhs=_r(xT[:, eb, tc4 * 512:(tc4 + 1) * 512]),
                                start=(eb == 0),
                                stop=(eb == NE - 1),
                            )
                    for i in range(2):
                        tc4 = tp * 2 + i
                        nc.scalar.activation(
                            _r(dstT[:, a, tc4 * 512:(tc4 + 1) * 512]), pts[i][:],
                            AF.Identity, bias=b_sb[:, a:a + 1],
                        )

            def project_v(a, xeT, vttp, ps1, only_tp=None):
                # V^T block a, then transpose into Vt [t, d']
                vtt = vttp.tile([128, T], F32, tag="vtt", name=f"vtt{a}_{only_tp}")
                for tp in ((only_tp,) if only_tp is not None else (0, 1)):
                    pts = [ps1.tile([128, 512], F32, tag="s", name=f"pv{a}_{tp}_{_i}")
                           for _i in range(2)]
                    for eb in range(NE):
                        for i in range(2):
                            tc4 = tp * 2 + i
                            nc.tensor.matmul(
                                pts[i][:],
                                lhsT=_r(wv_sb[:, eb, a * 128:(a + 1) * 128]),
                                rhs=_r(xeT[:, eb, tc4 * 512:(tc4 + 1) * 512]),
                                start=(eb == 0),
                                stop=(eb == NE - 1),
                            )
                    for i in range(2):
                        tc4 = tp * 2 + i
                        nc.scalar.activation(
                            vtt[:, tc4 * 512:(tc4 + 1) * 512], pts[i][:],
                            AF.Identity, bias=bv_sb[:, a:a + 1],
                        )
                for tg in ((only_tp,) if only_tp is not None else (0, 1)):
                    pt = ps1.tile([128, 1024], F32, tag="s", name=f"vt{a}_{tg}")
                    for j in range(NE):
                        tt = tg * NE + j
                        nc.tensor.transpose(pt[:, j * 128:(j + 1) * 128],
                                            vtt[:, tt * 128:(tt + 1) * 128], ident[:])
                    nc.vector.tensor_copy(
                        _r(Vt[:, tg * NE:(tg + 1) * NE, a * 128:(a + 1) * 128]),
                        pt[:].rearrange("p (b n) -> p b n", b=NE))

            # ================= phase 2: one head PAIR =====================
            def attention_pair(hb, pss, pso, pbufp, smallp, kpp, vmvp, fillers=None):
                # Heads (2*hb, 2*hb+1) processed together with FULL-array
                # matmuls: per 64-key subtile kts, a block-diagonal K
                # stationary [128 d-pair, 128 (head,key)-slots] yields both
                # heads' scores in one K=128 matmul; a block V' stationary
                # [128 (head,key), 128 d-pair] does attnV the same way.
                # Key slots: head-even -> partitions pi(kts) (the partition
                # range where V[t] for this subtile lives), head-odd -> the
                # complement (its V slice is DMA-moved across partitions).
                po = pso.tile([128, T], F32, tag="po", name=f"po{hb}")
                he, hod = 2 * hb, 2 * hb + 1
                prev = None  # deferred attnV of the previous subtile
                for kts in range(2 * NT):
                    tt, pi = kts // 2, 64 * (kts % 2)
                    pj = 64 - pi  # complement offset
                    t0 = kts * 64 % 128
                    # --- block-diagonal K stationary ---
                    kp = kpp.tile([128, 128], F32, tag="kp", name=f"kp{hb}_{kts}")
                    nc.vector.tensor_copy(_r(kp[0:64, pj:pj + 64]), zeros[0:64, :])
                    nc.vector.tensor_copy(_r(kp[64:128, pi:pi + 64]), zeros[64:128, :])
                    nc.vector.tensor_copy(_r(kp[0:64, pi:pi + 64]),
                                          KT[0:64, hb, kts * 64:(kts + 1) * 64])
                    nc.vector.tensor_copy(_r(kp[64:128, pj:pj + 64]),
                                          KT[64:128, hb, kts * 64:(kts + 1) * 64])
                    # --- V slice for the odd head moved to the complement ---
                    vmv = vmvp.tile([128, 64], F32, tag="vmv", name=f"vm{hb}_{kts}")
                    nc.sync.dma_start(out=vmv[pj:pj + 64, :],
                                      in_=Vt[t0:t0 + 64, tt, hod * 64:hod * 64 + 64])
                    # --- scores + exp (both heads per instruction) ---
                    pP = pbufp.tile([128, T], BF16, tag="P", name=f"pP{hb}_{kts}")
                    sums = smallp.tile([128, 2], F32, tag="sums", name=f"sm{hb}_{kts}")
                    for half in range(2):
                        ps = pss.tile([128, 1024], F32, tag="s", name=f"ps{hb}_{kts}_{half}")
                        for qc in range(2):
                            q0 = half * 1024 + qc * 512
                            nc.tensor.matmul(
                                ps[:, qc * 512:(qc + 1) * 512],
                                lhsT=_r(kp[:]),
                                rhs=_r(QT[:, hb, q0:q0 + 512]),
                                start=True, stop=True,
                            )
                        nc.scalar.activation(
                            pP[:, half * 1024:(half + 1) * 1024], ps[:],
                            AF.Exp, scale=0.125,
                            accum_out=sums[:, half:half + 1],
                        )
                    # deferred attnV of the previous subtile keeps the PE busy
                    if prev is not None:
                        emit_attnv(*prev)
                    # --- 1/sumexp, block V' stationary (bf16) ---
                    stot = smallp.tile([128, 1], F32, tag="stot", name=f"st{hb}_{kts}")
                    nc.vector.tensor_add(stot[:], sums[:, 0:1], sums[:, 1:2])
                    rec = smallp.tile([128, 1], F32, tag="rec", name=f"rc{hb}_{kts}")
                    nc.vector.reciprocal(rec[:], stot[:])
                    vp = smallp.tile([128, 128], BF16, tag="vp", name=f"vp{hb}_{kts}")
                    nc.gpsimd.memset(vp[pi:pi + 64, 64:128], 0.0)
                    nc.gpsimd.memset(vp[pj:pj + 64, 0:64], 0.0)
                    nc.vector.tensor_scalar_mul(
                        vp[pi:pi + 64, 0:64],
                        Vt[t0:t0 + 64, tt, he * 64:he * 64 + 64],
                        rec[pi:pi + 64, :])
                    nc.vector.tensor_scalar_mul(
                        vp[pj:pj + 64, 64:128], vmv[pj:pj + 64, :],
                        rec[pj:pj + 64, :])
                    prev = (po, vp, pP, kts)
                    if fillers and kts in fillers:
                        fillers[kts]()
                emit_attnv(*prev)
                nc.vector.tensor_copy(_r(OT[:, hb, :]), po[:])

            def emit_attnv(po, vp, pP, kts):
                for qc4 in range(4):
                    nc.tensor.matmul(
                        po[:, qc4 * 512:(qc4 + 1) * 512],
                        lhsT=vp[:],
                        rhs=pP[:, qc4 * 512:(qc4 + 1) * 512],
                        start=(kts == 0), stop=(kts == 2 * NT - 1),
                    )

            # ================= program order ==============================
            with (
                tc.tile_pool(name="xtp", bufs=1) as xtp,
                tc.tile_pool(name="vtt", bufs=1) as vttp,
                tc.tile_pool(name="ps_s", bufs=2, space="PSUM") as pss,
                tc.tile_pool(name="ps_o", bufs=1, space="PSUM") as pso,
                tc.tile_pool(name="pbuf", bufs=3) as pbufp,
                tc.tile_pool(name="small", bufs=4) as smallp,
                tc.tile_pool(name="kpp", bufs=3) as kpp,
                tc.tile_pool(name="vmv", bufs=4) as vmvp,
            ):
                with (
                    tc.tile_pool(name="xrow", bufs=2) as xrowp,
                    tc.tile_pool(name="wtmp", bufs=2) as wtmpp,
                ):
                    xT = xtp.tile([128, NE, T], F32, tag="xt")
                    load_weight(wq_d, wq_sb, NE, DL, wtmpp)
                    transpose_in(x_d, xT, xrowp, pss)
                    load_weight(wk_d, wk_sb, NE, DL, wtmpp)
                    load_weight(wv_d, wv_sb, NE, DL, wtmpp)
                    project(wq_sb, bq_sb, xT, QT, 0, pss)
                    project(wq_sb, bq_sb, xT, QT, 1, pss)

                    xeT = xtp.tile([128, NE, T], F32, tag="xt")
                    transpose_in(xe_d, xeT, xrowp, pss)
                    load_weight(wo_d, wo_sb, 2, E, wtmpp)
                    botmp = wtmpp.tile([128, E], F32, tag="wtmp")
                    nc.sync.dma_start(out=botmp[0:1, :], in_=bo_d[:].rearrange("(o e) -> o e", o=1))
                    nc.vector.tensor_copy(_r(bo_r[0:1, :]), botmp[0:1, :])
                    project(wk_sb, bk_sb, xeT, KT, 0, pss)
                    project_v(0, xeT, vttp, pss)
                    # pair 0 attention; the tail of pair 0 interleaves the
                    # first half of the a=1 projections as fillers, the rest
                    # lands inside pair 1's early iterations
                    attention_pair(0, pss, pso, pbufp, smallp, kpp, vmvp, fillers={
                        19: lambda: project(wk_sb, bk_sb, xeT, KT, 1, pss, only_tp=0),
                        25: lambda: project_v(1, xeT, vttp, pss, only_tp=0),
                    })
                    attention_pair(1, pss, pso, pbufp, smallp, kpp, vmvp, fillers={
                        3: lambda: project(wk_sb, bk_sb, xeT, KT, 1, pss, only_tp=1),
                        9: lambda: project_v(1, xeT, vttp, pss, only_tp=1),
                    })

            # ================= phase 3: output projection =================
            with (
                tc.tile_pool(name="ps_y", bufs=4, space="PSUM") as psy,
                tc.tile_pool(name="ysb", bufs=2) as ysbp,
            ):
                for tt in range(NT):
                    ysb = ysbp.tile([128, E], F32, tag="ysb")
                    pys = [psy.tile([128, 512], F32, tag="py", name=f"py{tt}_{_i}")
                           for _i in range(2)]
                    for a in range(2):
                        for ec in range(2):
                            nc.tensor.matmul(
                                pys[ec][:],
                                lhsT=_r(OT[:, a, tt * 128:(tt + 1) * 128]),
                                rhs=_r(wo_sb[:, a, ec * 512:(ec + 1) * 512]),
                                start=(a == 0), stop=False,
                            )
                    for ec in range(2):
                        nc.tensor.matmul(
                            pys[ec][:],
                            lhsT=_r(ones[0:1, 0:128]),
                            rhs=_r(bo_r[0:1, ec * 512:(ec + 1) * 512]),
                            start=False, stop=True,
                        )
                        nc.vector.tensor_copy(ysb[:, ec * 512:(ec + 1) * 512], pys[ec][:])
                    nc.sync.dma_start(out=y_d[tt * 128:(tt + 1) * 128, :], in_=ysb[:])

    nc.compile()
    return nc


_NC_CACHE = []


def _get_program():
    if not _NC_CACHE:
        _NC_CACHE.append(build_program())
    return _NC_CACHE[0]


def make_in_maps(input, encoded_features, Wq, bq, Wkv, bkv, Wo, bo):
    input = np.asarray(input, dtype=np.float32)
    encoded_features = np.asarray(encoded_features, dtype=np.float32)
    Wq = np.asarray(Wq, dtype=np.float32)
    bq = np.asarray(bq, dtype=np.float32)
    Wkv = np.asarray(Wkv, dtype=np.float32)
    bkv = np.asarray(bkv, dtype=np.float32)
    Wo = np.asarray(Wo, dtype=np.float32)
    bo = np.asarray(bo, dtype=np.float32)
    in_maps = []
    for c in range(NCORES):
        b, hg = c // 4, c % 4
        s = slice(hg * DL, (hg + 1) * DL)
        in_maps.append({
            "x": np.ascontiguousarray(input[b]),
            "xe": np.ascontiguousarray(encoded_features[b]),
            "wq": np.ascontiguousarray(Wq[:, s]),
            "wk": np.ascontiguousarray(Wkv[:, s]),
            "wv": np.ascontiguousarray(Wkv[:, E + hg * DL:E + (hg + 1) * DL]),
            "wo": np.ascontiguousarray(Wo[s, :]),
            "bq": np.ascontiguousarray(bq[s]),
            "bk": np.ascontiguousarray(bkv[s]),
            "bv": np.ascontiguousarray(bkv[E + hg * DL:E + (hg + 1) * DL]),
            "bo": (bo if hg == 0 else np.zeros_like(bo)),
        })
    return in_maps


def combine_outputs(results):
    out = np.zeros((B, T, E), dtype=np.float32)
    for c in range(NCORES):
        out[c // 4] += results[c]["y"]
    return out


def kernel(input, encoded_features, Wq, bq, Wkv, bkv, Wo, bo, _trace=False):
    nc = _get_program()
    in_maps = make_in_maps(input, encoded_features, Wq, bq, Wkv, bkv, Wo, bo)
    res = run_bass_kernel_spmd(nc, in_maps, list(range(NCORES)), trace=_trace)
    out = combine_outputs(res.results)
    if _trace:
        kernel.last_exec_time_ns = res.exec_time_ns
        kernel.last_results = res
    return out


if __name__ == "__main__":
    rng = np.random.default_rng(0)
    inputs = {
        "input": rng.standard_normal((B, T, E), dtype=np.float32),
        "encoded_features": rng.standard_normal((B, T, E), dtype=np.float32),
        "Wq": (rng.standard_normal((E, E), dtype=np.float32) / 32.0),
        "bq": np.zeros(E, np.float32),
        "Wkv": (rng.standard_normal((E, 2 * E), dtype=np.float32) / 32.0),
        "bkv": np.zeros(2 * E, np.float32),
        "Wo": (rng.standard_normal((E, E), dtype=np.float32) / 32.0),
        "bo": np.zeros(E, np.float32),
    }
    out = kernel(**inputs)
    print("out shape", out.shape, out.dtype)


# revision 21
# speedup vs baseline: 1.2194x; 1.0394x over previous
"""Multi-head cross-attention (softmax over the QUERY axis) on 8 TRN2 cores.

Sharding: core c handles batch b = c // 4 and head-group hg = c % 4
(4 heads of 64 dims each = 256 columns of Wq / Wkv-K / Wkv-V, 256 rows
of Wo).  Each core computes a partial final output (its head group's
contribution to  out @ Wo); the host sums the 4 partials per batch.
bo is fed only to the hg == 0 cores so it is added exactly once.

Per-core device kernel (single Bass SPMD program, data-driven):
  phase 1: transpose X / Xe on the PE (exact transpose mode), project
           Q^T, K^T  [256, 2048]  and V [2048, 256]  with fp32r matmuls
  phase 2: heads processed in PAIRS sharing the PE array via tile
           position inference (even head: SBUF partitions 0-63 -> T0,
           odd head: partitions 64-127 -> T8 for the 64-contraction
           score matmuls; attnV col-tiles T0/T1 write the two PSUM
           partition halves of one [128, 2048] out^T accumulator).
           scores S^T[k, q]; one ACT Exp pass evacuates PSUM -> SBUF
           and emits per-key sum(exp) via accum_out; 1/sum is folded
           into V (per-partition scale).
  phase 3: Y[t, :] = O^T.T @ Wo (+ bo via a K=1 ones-row matmul)
"""

import os
import sys

for _p in ("/opt/trn_rl_repo", "/root/.axon_site/_ro/trn_rl_repo"):
    if os.path.isdir(_p) and _p not in sys.path:
        sys.path.insert(0, _p)

import numpy as np

import concourse.bass as bass  # noqa: E402
import concourse.mybir as mybir  # noqa: E402
import concourse.tile as tile  # noqa: E402
from concourse import bacc  # noqa: E402
from concourse.bass_utils import run_bass_kernel_spmd  # noqa: E402
from concourse.masks import make_identity  # noqa: E402

B, T, E, H, D = 2, 2048, 1024, 16, 64
HG = 4                # heads per core
DL = HG * D           # 256 local head-dim columns
NCORES = 8
NT = T // 128         # 16 row tiles
NE = E // 128         # 8 contraction blocks
F32 = mybir.dt.float32
F32R = mybir.dt.float32r
BF16 = mybir.dt.bfloat16
AF = mybir.ActivationFunctionType


def _r(ap):
    return ap.bitcast(F32R)


def build_program():
    nc = bacc.Bacc()
    x_d = nc.dram_tensor("x", [T, E], F32, kind="ExternalInput")
    xe_d = nc.dram_tensor("xe", [T, E], F32, kind="ExternalInput")
    wq_d = nc.dram_tensor("wq", [E, DL], F32, kind="ExternalInput")
    wk_d = nc.dram_tensor("wk", [E, DL], F32, kind="ExternalInput")
    wv_d = nc.dram_tensor("wv", [E, DL], F32, kind="ExternalInput")
    wo_d = nc.dram_tensor("wo", [DL, E], F32, kind="ExternalInput")
    bq_d = nc.dram_tensor("bq", [DL], F32, kind="ExternalInput")
    bk_d = nc.dram_tensor("bk", [DL], F32, kind="ExternalInput")
    bv_d = nc.dram_tensor("bv", [DL], F32, kind="ExternalInput")
    bo_d = nc.dram_tensor("bo", [E], F32, kind="ExternalInput")
    y_d = nc.dram_tensor("y", [T, E], F32, kind="ExternalOutput")

    with tile.TileContext(nc) as tc:
        with (
            tc.tile_pool(name="const", bufs=1) as cons,
            tc.tile_pool(name="persist", bufs=1) as pers,
            tc.tile_pool(name="wqkv", bufs=1) as wqkvp,
        ):
            # ---- constants first (gpsimd), weights later on the sync q ---
            ones = cons.tile([128, 128], F32, tag="ones")
            nc.gpsimd.memset(ones[:], 1.0)
            nc.vector.tensor_copy(_r(ones[:]), ones[:])
            ident = cons.tile([128, 128], F32, tag="ident")
            make_identity(nc, ident[:])
            zeros = cons.tile([128, 64], F32, tag="zeros")
            nc.gpsimd.memset(zeros[:], 0.0)
            bq_sb = cons.tile([128, 2], F32, tag="bq")
            bk_sb = cons.tile([128, 2], F32, tag="bk")
            bv_sb = cons.tile([128, 2], F32, tag="bv")
            nc.gpsimd.dma_start(out=bq_sb[:], in_=bq_d[:].rearrange("(a p) -> p a", p=128))
            nc.gpsimd.dma_start(out=bk_sb[:], in_=bk_d[:].rearrange("(a p) -> p a", p=128))
            nc.gpsimd.dma_start(out=bv_sb[:], in_=bv_d[:].rearrange("(a p) -> p a", p=128))
            wq_sb = wqkvp.tile([128, NE, DL], F32, tag="wq")
            wk_sb = wqkvp.tile([128, NE, DL], F32, tag="wk")
            wv_sb = wqkvp.tile([128, NE, DL], F32, tag="wv")
            wo_sb = cons.tile([128, 2, E], F32, tag="wo")
            bo_r = cons.tile([128, E], F32, tag="bo_r")

            def load_weight(w_d, w_sb, nb, nn, wtmpp):
                wr = w_d[:].rearrange("(a p) n -> p a n", p=128)
                for blk in range(nb):
                    wtmp = wtmpp.tile([128, E], F32, tag="wtmp", name=f"wt{blk}")
                    nc.sync.dma_start(out=wtmp[:, 0:nn], in_=wr[:, blk, :])
                    nc.vector.tensor_copy(_r(w_sb[:, blk, :]), wtmp[:, 0:nn])

            # ---- persistent activations ----------------------------------
            QT = pers.tile([128, 2, T], F32, tag="QT")   # Q^T: part = d' % 128, blk = d' // 128
            KT = pers.tile([128, 2, T], F32, tag="KT")
            Vt = pers.tile([128, NT, DL], F32, tag="V")  # V:   part = t % 128, blk = t // 128
            OT = pers.tile([128, 2, T], F32, tag="OT")   # out^T, same layout as QT

            # ================= phase 1 helpers ============================
            def transpose_in(src_d, xT, xrowp, ps1):
                for tt in range(NT):
                    xrow = xrowp.tile([128, E], F32, tag="xrow")
                    nc.sync.dma_start(out=xrow[:], in_=src_d[tt * 128:(tt + 1) * 128, :])
                    pt = ps1.tile([128, 1024], F32, tag="s", name=f"tp{tt}")
                    for eb in range(NE):
                        nc.tensor.transpose(pt[:, eb * 128:(eb + 1) * 128],
                                            xrow[:, eb * 128:(eb + 1) * 128], ident[:])
                    dst = _r(xT[:, 0:NE, tt * 128:(tt + 1) * 128])
                    if tt % 2 == 0:
                        nc.vector.tensor_copy(dst, pt[:].rearrange("p (b n) -> p b n", b=NE))
                    else:
                        nc.scalar.activation(dst, pt[:].rearrange("p (b n) -> p b n", b=NE), AF.Copy)

            def project(w_sb, b_sb, xT, dstT, a, ps1, only_tp=None):
                # dstT[:, a, t] = sum_eb w[:, eb, a*128:+128].T @ xT[:, eb, t] + b
                for tp in ((only_tp,) if only_tp is not None else (0, 1)):
                    pts = [ps1.tile([128, 512], F32, tag="s", name=f"pt{a}_{tp}_{_i}")
                           for _i in range(2)]
                    for eb in range(NE):
                        for i in range(2):
                            tc4 = tp * 2 + i
                            nc.tensor.matmul(
                                pts[i][:],
                                lhsT=_r(w_sb[:, eb, a * 128:(a + 1) * 128]),
                                rhs=_r(xT[:, eb, tc4 * 512:(tc4 + 1) * 512]),
                                start=(eb == 0),
                                stop=(eb == NE - 1),
                            )
                    for i in range(2):
                        tc4 = tp * 2 + i
                        nc.scalar.activation(
                            _r(dstT[:, a, tc4 * 512:(tc4 + 1) * 512]), pts[i][:],
                            AF.Identity, bias=b_sb[:, a:a + 1],
                        )

            def project_v(a, xeT, vttp, ps1, only_tp=None):
                # V^T block a, then transpose into Vt [t, d']
                vtt = vttp.tile([128, T], F32, tag="vtt", name=f"vtt{a}_{only_tp}")
                for tp in ((only_tp,) if only_tp is not None else (0, 1)):
                    pts = [ps1.tile([128, 512], F32, tag="s", name=f"pv{a}_{tp}_{_i}")
                           for _i in range(2)]
                    for eb in range(NE):
                        for i in range(2):
                            tc4 = tp * 2 + i
                            nc.tensor.matmul(
                                pts[i][:],
                                lhsT=_r(wv_sb[:, eb, a * 128:(a + 1) * 128]),
                                rhs=_r(xeT[:, eb, tc4 * 512:(tc4 + 1) * 512]),
                                start=(eb == 0),
                                stop=(eb == NE - 1),
                            )
                    for i in range(2):
                        tc4 = tp * 2 + i
                        nc.scalar.activation(
                            vtt[:, tc4 * 512:(tc4 + 1) * 512], pts[i][:],
                            AF.Identity, bias=bv_sb[:, a:a + 1],
                        )
                for tg in ((only_tp,) if only_tp is not None else (0, 1)):
                    pt = ps1.tile([128, 1024], F32, tag="s", name=f"vt{a}_{tg}")
                    for j in range(NE):
                        tt = tg * NE + j
                        nc.tensor.transpose(pt[:, j * 128:(j + 1) * 128],
                                            vtt[:, tt * 128:(tt + 1) * 128], ident[:])
                    nc.vector.tensor_copy(
                        _r(Vt[:, tg * NE:(tg + 1) * NE, a * 128:(a + 1) * 128]),
                        pt[:].rearrange("p (b n) -> p b n", b=NE))

            # ================= phase 2: one head PAIR =====================
            def attention_pair(hb, pss, pso, pbufp, smallp, kpp, vmvp, fillers=None):
                # Heads (2*hb, 2*hb+1) processed together with FULL-array
                # matmuls: per 64-key subtile kts, a block-diagonal K
                # stationary [128 d-pair, 128 (head,key)-slots] yields both
                # heads' scores in one K=128 matmul; a block V' stationary
                # [128 (head,key), 128 d-pair] does attnV the same way.
                # Key slots: head-even -> partitions pi(kts) (the partition
                # range where V[t] for this subtile lives), head-odd -> the
                # complement (its V slice is DMA-moved across partitions).
                po = pso.tile([128, T], F32, tag="po", name=f"po{hb}")
                he, hod = 2 * hb, 2 * hb + 1
                prev = None  # deferred attnV of the previous subtile
                for kts in range(2 * NT):
                    tt, pi = kts // 2, 64 * (kts % 2)
                    pj = 64 - pi  # complement offset
                    t0 = kts * 64 % 128
                    # --- block-diagonal K stationary ---
                    kp = kpp.tile([128, 128], F32, tag="kp", name=f"kp{hb}_{kts}")
                    nc.vector.tensor_copy(_r(kp[0:64, pj:pj + 64]), zeros[0:64, :])
                    nc.vector.tensor_copy(_r(kp[64:128, pi:pi + 64]), zeros[64:128, :])
                    nc.vector.tensor_copy(_r(kp[0:64, pi:pi + 64]),
                                          KT[0:64, hb, kts * 64:(kts + 1) * 64])
                    nc.vector.tensor_copy(_r(kp[64:128, pj:pj + 64]),
                                          KT[64:128, hb, kts * 64:(kts + 1) * 64])
                    # --- V slice for the odd head moved to the complement ---
                    vmv = vmvp.tile([128, 64], F32, tag="vmv", name=f"vm{hb}_{kts}")
                    nc.sync.dma_start(out=vmv[pj:pj + 64, :],
                                      in_=Vt[t0:t0 + 64, tt, hod * 64:hod * 64 + 64])
                    # --- scores + exp (both heads per instruction) ---
                    pP = pbufp.tile([128, T], BF16, tag="P", name=f"pP{hb}_{kts}")
                    sums = smallp.tile([128, 2], F32, tag="sums", name=f"sm{hb}_{kts}")
                    for half in range(2):
                        ps = pss.tile([128, 1024], F32, tag="s", name=f"ps{hb}_{kts}_{half}")
                        for qc in range(2):
                            q0 = half * 1024 + qc * 512
                            nc.tensor.matmul(
                                ps[:, qc * 512:(qc + 1) * 512],
                                lhsT=_r(kp[:]),
                                rhs=_r(QT[:, hb, q0:q0 + 512]),
                                start=True, stop=True,
                            )
                        nc.scalar.activation(
                            pP[:, half * 1024:(half + 1) * 1024], ps[:],
                            AF.Exp, scale=0.125,
                            accum_out=sums[:, half:half + 1],
                        )
                    # deferred attnV of the previous subtile keeps the PE busy
                    if prev is not None:
                        emit_attnv(*prev)
                    # --- 1/sumexp, block V' stationary (bf16) ---
                    stot = smallp.tile([128, 1], F32, tag="stot", name=f"st{hb}_{kts}")
                    nc.vector.tensor_add(stot[:], sums[:, 0:1], sums[:, 1:2])
                    rec = smallp.tile([128, 1], F32, tag="rec", name=f"rc{hb}_{kts}")
                    nc.vector.reciprocal(rec[:], stot[:])
                    vp = smallp.tile([128, 128], BF16, tag="vp", name=f"vp{hb}_{kts}")
                    nc.gpsimd.memset(vp[pi:pi + 64, 64:128], 0.0)
                    nc.gpsimd.memset(vp[pj:pj + 64, 0:64], 0.0)
                    nc.vector.tensor_scalar_mul(
                        vp[pi:pi + 64, 0:64],
                        Vt[t0:t0 + 64, tt, he * 64:he * 64 + 64],
                        rec[pi:pi + 64, :])
                    nc.vector.tensor_scalar_mul(
                        vp[pj:pj + 64, 64:128], vmv[pj:pj + 64, :],
                        rec[pj:pj + 64, :])
                    prev = (po, vp, pP, kts)
                    if fillers and kts in fillers:
                        fillers[kts]()
                emit_attnv(*prev)
                nc.vector.tensor_copy(_r(OT[:, hb, :]), po[:])

            def emit_attnv(po, vp, pP, kts):
                for qc4 in range(4):
                    nc.tensor.matmul(
                        po[:, qc4 * 512:(qc4 + 1) * 512],
                        lhsT=vp[:],
                        rhs=pP[:, qc4 * 512:(qc4 + 1) * 512],
                        start=(kts == 0), stop=(kts == 2 * NT - 1),
                    )

            # ================= program order ==============================
            with (
                tc.tile_pool(name="xtp", bufs=1) as xtp,
                tc.tile_pool(name="vtt", bufs=1) as vttp,
                tc.tile_pool(name="pbuf", bufs=3) as pbufp,
                tc.tile_pool(name="small", bufs=4) as smallp,
                tc.tile_pool(name="kpp", bufs=3) as kpp,
                tc.tile_pool(name="vmv", bufs=4) as vmvp,
            ):
                xT = xtp.tile([128, NE, T], F32, tag="xt")
                with (
                    tc.tile_pool(name="xrow", bufs=2) as xrowp,
                    tc.tile_pool(name="wtmp", bufs=2) as wtmpp,
                    tc.tile_pool(name="ph1", bufs=4, space="PSUM") as ph1,
                ):
                    load_weight(wq_d, wq_sb, NE, DL, wtmpp)
                    transpose_in(x_d, xT, xrowp, ph1)
                    load_weight(wk_d, wk_sb, NE, DL, wtmpp)
                    load_weight(wv_d, wv_sb, NE, DL, wtmpp)
                    project(wq_sb, bq_sb, xT, QT, 0, ph1)
                    project(wq_sb, bq_sb, xT, QT, 1, ph1)

                    xeT = xtp.tile([128, NE, T], F32, tag="xt")
                    transpose_in(xe_d, xeT, xrowp, ph1)
                    load_weight(wo_d, wo_sb, 2, E, wtmpp)
                    botmp = wtmpp.tile([128, E], F32, tag="wtmp")
                    nc.sync.dma_start(out=botmp[0:1, :], in_=bo_d[:].rearrange("(o e) -> o e", o=1))
                    nc.vector.tensor_copy(_r(bo_r[0:1, :]), botmp[0:1, :])
                    project(wk_sb, bk_sb, xeT, KT, 0, ph1)
                    project_v(0, xeT, vttp, ph1)
                with (
                    tc.tile_pool(name="ps_s", bufs=2, space="PSUM") as pss,
                    tc.tile_pool(name="ps_o", bufs=1, space="PSUM") as pso,
                ):
                    # pair 0 attention; the a=1 projections ride along as
                    # fillers at the tail of pair 0 / head of pair 1
                    attention_pair(0, pss, pso, pbufp, smallp, kpp, vmvp, fillers={
                        26: lambda: project(wk_sb, bk_sb, xeT, KT, 1, pss, only_tp=0),
                        29: lambda: project_v(1, xeT, vttp, pss, only_tp=0),
                    })
                    attention_pair(1, pss, pso, pbufp, smallp, kpp, vmvp, fillers={
                        1: lambda: project(wk_sb, bk_sb, xeT, KT, 1, pss, only_tp=1),
                        5: lambda: project_v(1, xeT, vttp, pss, only_tp=1),
                    })

            # ================= phase 3: output projection =================
            with (
                tc.tile_pool(name="ps_y", bufs=6, space="PSUM") as psy,
                tc.tile_pool(name="ysb", bufs=2) as ysbp,
            ):
                for tt in range(NT):
                    ysb = ysbp.tile([128, E], F32, tag="ysb")
                    pys = [psy.tile([128, 512], F32, tag="py", name=f"py{tt}_{_i}")
                           for _i in range(2)]
                    for a in range(2):
                        for ec in range(2):
                            nc.tensor.matmul(
                                pys[ec][:],
                                lhsT=_r(OT[:, a, tt * 128:(tt + 1) * 128]),
                                rhs=_r(wo_sb[:, a, ec * 512:(ec + 1) * 512]),
                                start=(a == 0), stop=False,
                            )
                    for ec in range(2):
                        nc.tensor.matmul(
                            pys[ec][:],
                            lhsT=_r(ones[0:1, 0:128]),
                            rhs=_r(bo_r[0:1, ec * 512:(ec + 1) * 512]),
                            start=False, stop=True,
                        )
                        nc.scalar.activation(ysb[:, ec * 512:(ec + 1) * 512], pys[ec][:], AF.Copy)
                    nc.sync.dma_start(out=y_d[tt * 128:(tt + 1) * 128, :], in_=ysb[:])

    nc.compile()
    return nc


_NC_CACHE = []


def _get_program():
    if not _NC_CACHE:
        _NC_CACHE.append(build_program())
    return _NC_CACHE[0]


def make_in_maps(input, encoded_features, Wq, bq, Wkv, bkv, Wo, bo):
    input = np.asarray(input, dtype=np.float32)
    encoded_features = np.asarray(encoded_features, dtype=np.float32)
    Wq = np.asarray(Wq, dtype=np.float32)
    bq = np.asarray(bq, dtype=np.float32)
    Wkv = np.asarray(Wkv, dtype=np.float32)
    bkv = np.asarray(bkv, dtype=np.float32)
    Wo = np.asarray(Wo, dtype=np.float32)
    bo = np.asarray(bo, dtype=np.float32)
    in_maps = []
    for c in range(NCORES):
        b, hg = c // 4, c % 4
        s = slice(hg * DL, (hg + 1) * DL)
        in_maps.append({
            "x": np.ascontiguousarray(input[b]),
            "xe": np.ascontiguousarray(encoded_features[b]),
            "wq": np.ascontiguousarray(Wq[:, s]),
            "wk": np.ascontiguousarray(Wkv[:, s]),
            "wv": np.ascontiguousarray(Wkv[:, E + hg * DL:E + (hg + 1) * DL]),
            "wo": np.ascontiguousarray(Wo[s, :]),
            "bq": np.ascontiguousarray(bq[s]),
            "bk": np.ascontiguousarray(bkv[s]),
            "bv": np.ascontiguousarray(bkv[E + hg * DL:E + (hg + 1) * DL]),
            "bo": (bo if hg == 0 else np.zeros_like(bo)),
        })
    return in_maps


def combine_outputs(results):
    out = np.zeros((B, T, E), dtype=np.float32)
    for c in range(NCORES):
        out[c // 4] += results[c]["y"]
    return out


def kernel(input, encoded_features, Wq, bq, Wkv, bkv, Wo, bo, _trace=False):
    nc = _get_program()
    in_maps = make_in_maps(input, encoded_features, Wq, bq, Wkv, bkv, Wo, bo)
    res = run_bass_kernel_spmd(nc, in_maps, list(range(NCORES)), trace=_trace)
    out = combine_outputs(res.results)
    if _trace:
        kernel.last_exec_time_ns = res.exec_time_ns
        kernel.last_results = res
    return out


if __name__ == "__main__":
    rng = np.random.default_rng(0)
    inputs = {
        "input": rng.standard_normal((B, T, E), dtype=np.float32),
        "encoded_features": rng.standard_normal((B, T, E), dtype=np.float32),
        "Wq": (rng.standard_normal((E, E), dtype=np.float32) / 32.0),
        "bq": np.zeros(E, np.float32),
        "Wkv": (rng.standard_normal((E, 2 * E), dtype=np.float32) / 32.0),
        "bkv": np.zeros(2 * E, np.float32),
        "Wo": (rng.standard_normal((E, E), dtype=np.float32) / 32.0),
        "bo": np.zeros(E, np.float32),
    }
    out = kernel(**inputs)
    print("out shape", out.shape, out.dtype)
